# revision 12
# baseline (speedup 1.0000x reference)
"""Multi-head attention (B=2, S=4096, D=512, H=8) on 8 TRN2 NeuronCores.

Sharding: batch x sequence. Core c handles batch b=c//4, token slice
s=c%4 (1024 tokens). Each core projects the full K/V for its batch
(replicated within the 4-core batch group), projects Q for its token
slice, runs flash-style attention (scores kept transposed [tk, tq] so
no on-chip transposes are needed), and applies the output projection
for its tokens. Gather = pure concatenation, no reduction.

All matmuls in bf16 with fp32 PSUM accumulation. Softmax skips the
max-subtraction (scores ~ N(0,1); max < ~6, exp is safe in fp32) and
the denominator is obtained by augmenting V with a ones column, so
softmax costs exactly one ACT pass over the scores.
"""

import numpy as np
import ml_dtypes

B, S, D = 2, 4096, 512
H, DK = 8, 64
N_CORES = 8
TOK = B * S * DK // (N_CORES * DK)  # 1024 tokens per core
TOK = 1024

_PROGRAM = None


def _build_program():
    from contextlib import ExitStack

    import concourse.bass as bass  # noqa: F401
    import concourse.mybir as mybir
    import concourse.tile as tile
    from concourse import bacc

    bf = mybir.dt.bfloat16
    f32 = mybir.dt.float32
    Exp = mybir.ActivationFunctionType.Exp

    nc = bacc.Bacc(None)

    qT = nc.declare_dram_parameter("qT", [D, TOK], bf, isOutput=False)
    kT = nc.declare_dram_parameter("kT", [D, S], bf, isOutput=False)
    vT = nc.declare_dram_parameter("vT", [D, S], bf, isOutput=False)
    wqT = nc.declare_dram_parameter("wqT", [D, D], bf, isOutput=False)
    wkT = nc.declare_dram_parameter("wkT", [D, D], bf, isOutput=False)
    wvT = nc.declare_dram_parameter("wvT", [D, D], bf, isOutput=False)
    woT = nc.declare_dram_parameter("woT", [D, D], bf, isOutput=False)
    bq = nc.declare_dram_parameter("bq", [1, D], bf, isOutput=False)
    bk = nc.declare_dram_parameter("bk", [1, D], bf, isOutput=False)
    bv = nc.declare_dram_parameter("bv", [1, D], bf, isOutput=False)
    bo = nc.declare_dram_parameter("bo", [1, D], bf, isOutput=False)
    out_p = nc.declare_dram_parameter("out", [TOK, D], f32, isOutput=True)
    # DRAM scratch for broadcasting softmax reciprocal rows across partitions
    rscratch = nc.dram_tensor("rscratch", [H * 2, 512], f32)

    with tile.TileContext(nc) as tc, ExitStack() as ctx:
        wpool = ctx.enter_context(tc.tile_pool(name="w", bufs=1))
        kstream = ctx.enter_context(tc.tile_pool(name="kstream", bufs=2))
        qstream = ctx.enter_context(tc.tile_pool(name="qstream", bufs=2))
        ktres = ctx.enter_context(tc.tile_pool(name="ktres", bufs=4))
        qtres = ctx.enter_context(tc.tile_pool(name="qtres", bufs=4))
        vstore = ctx.enter_context(tc.tile_pool(name="vstore", bufs=32))
        ppool = ctx.enter_context(tc.tile_pool(name="p", bufs=3))
        opool = ctx.enter_context(tc.tile_pool(name="o", bufs=8))
        wspool = ctx.enter_context(tc.tile_pool(name="ws", bufs=2))
        ostage = ctx.enter_context(tc.tile_pool(name="ostage", bufs=2))
        projp = ctx.enter_context(tc.tile_pool(name="projp", bufs=2, space="PSUM"))
        scorep = ctx.enter_context(tc.tile_pool(name="scorep", bufs=2, space="PSUM"))
        pvp = ctx.enter_context(tc.tile_pool(name="pvp", bufs=2, space="PSUM"))

        dma = nc.sync.dma_start
        MM = nc.tensor.matmul

        # ---- constants: weights, biases, ones row ----
        def wtiles(param, tagp):
            t = wpool.tile([128, 4, D], bf, tag=tagp, name=tagp)
            dma(out=t[:], in_=param[:].rearrange("(c p) d -> p c d", p=128))
            return t

        wq_t = wtiles(wqT, "wq")
        wk_t = wtiles(wkT, "wk")
        wv_t = wtiles(wvT, "wv")
        wo_t = wpool.tile([DK, H, D], bf, tag="wo", name="wo_t")
        dma(out=wo_t[:], in_=woT[:].rearrange("(h p) d -> p h d", p=DK))
        bq_t = wpool.tile([1, D], bf, tag="bq", name="bq_t")
        dma(out=bq_t[:], in_=bq[:])
        bk_t = wpool.tile([1, D], bf, tag="bk", name="bk_t")
        dma(out=bk_t[:], in_=bk[:])
        bv_t = wpool.tile([1, D], bf, tag="bv", name="bv_t")
        dma(out=bv_t[:], in_=bv[:])
        bo_t = wpool.tile([1, D], bf, tag="bo", name="bo_t")
        dma(out=bo_t[:], in_=bo[:])
        ones1 = wpool.tile([1, D], bf, tag="ones", name="ones1")
        nc.vector.memset(ones1[:], 1.0)

        kt_res = []  # Q^T/K^T resident tiles, one per dout tile of 128
        qt_res = []
        v_store = []  # 32 tiles [128, H, DK+1]; last col per head = ones
        o_tiles = []  # 8 tiles [64, TOK] = normalized O_h^T

        def proj_dout(d):
            """Project Q^T and K^T for output-dim tile d (heads 2d, 2d+1)."""
            qraw = qstream.tile([128, 4, TOK], bf, tag="qraw", name="qraw")
            dma(out=qraw[:], in_=qT[:].rearrange("(c p) t -> p c t", p=128))
            qt = qtres.tile([128, TOK], bf, tag="qtres", name="qtres")
            qt_res.append(qt)
            for half in range(2):
                ps = projp.tile([128, 512], f32, tag="proj", name="proj_ps")
                for kk in range(4):
                    MM(
                        ps[:],
                        wq_t[:, kk, d * 128 : (d + 1) * 128],
                        qraw[:, kk, half * 512 : half * 512 + 512],
                        start=(kk == 0),
                        stop=False,
                        skip_group_check=True,
                    )
                MM(
                    ps[:],
                    bq_t[0:1, d * 128 : (d + 1) * 128],
                    ones1[0:1, :],
                    start=False,
                    stop=True,
                    skip_group_check=True,
                )
                nc.vector.tensor_copy(
                    out=qt[:, half * 512 : half * 512 + 512], in_=ps[:]
                )
            kt = ktres.tile([128, S], bf, tag="ktres", name="ktres")
            kt_res.append(kt)
            for tch in range(8):
                kraw = kstream.tile([128, 4, 512], bf, tag="kraw", name="kraw")
                dma(
                    out=kraw[:],
                    in_=kT[:, tch * 512 : (tch + 1) * 512].rearrange(
                        "(c p) t -> p c t", p=128
                    ),
                )
                ps = projp.tile([128, 512], f32, tag="proj", name="proj_ps")
                for kk in range(4):
                    MM(
                        ps[:],
                        wk_t[:, kk, d * 128 : (d + 1) * 128],
                        kraw[:, kk, :],
                        start=(kk == 0),
                        stop=False,
                        skip_group_check=True,
                    )
                MM(
                    ps[:],
                    bk_t[0:1, d * 128 : (d + 1) * 128],
                    ones1[0:1, :],
                    start=False,
                    stop=True,
                    skip_group_check=True,
                )
                nc.vector.tensor_copy(
                    out=kt[:, tch * 512 : (tch + 1) * 512], in_=ps[:]
                )

        vraw_cur = []

        def emit_v(j):
            """Project V for s-chunk j (tokens j*128..j*128+128)."""
            c, sub = divmod(j, 4)
            if sub == 0:
                vraw_cur.clear()
                t = kstream.tile([128, 4, 512], bf, tag="vraw", name="vraw")
                dma(
                    out=t[:],
                    in_=vT[:, c * 512 : (c + 1) * 512].rearrange(
                        "(c p) t -> p c t", p=128
                    ),
                )
                vraw_cur.append(t)
            ps = projp.tile([128, 512], f32, tag="proj", name="proj_ps")
            for kk in range(4):
                MM(
                    ps[:],
                    vraw_cur[0][:, kk, sub * 128 : (sub + 1) * 128],
                    wv_t[:, kk, :],
                    start=(kk == 0),
                    stop=False,
                    skip_group_check=True,
                )
            MM(
                ps[:],
                ones1[0:1, 0:128],
                bv_t[:],
                start=False,
                stop=True,
                skip_group_check=True,
            )
            vs = vstore.tile([128, H, DK + 1], bf, tag="vs", name="vs")
            v_store.append(vs)
            nc.vector.memset(vs[:, :, DK : DK + 1], 1.0)
            nc.vector.tensor_copy(
                out=vs[:, :, 0:DK],
                in_=ps[:].rearrange("p (h c) -> p h c", c=DK),
            )

        def head(h, interleave_v):
            kt = kt_res[h // 2]
            qt = qt_res[h // 2]
            pb = (h % 2) * 64
            pv = [pvp.tile([DK + 1, 512], f32, tag="pv", name=f"pv{_h}") for _h in range(2)]
            for j in range(32):
                if interleave_v:
                    emit_v(j)
                sc = scorep.tile([128, 1024], f32, tag="sc", name="sc")
                for half in range(2):
                    MM(
                        sc[:, half * 512 : half * 512 + 512],
                        kt[pb : pb + 64, j * 128 : (j + 1) * 128],
                        qt[pb : pb + 64, half * 512 : half * 512 + 512],
                        start=True,
                        stop=True,
                        skip_group_check=True,
                    )
                pt = ppool.tile([128, 1024], bf, tag="pt", name="pt")
                nc.scalar.activation(out=pt[:], in_=sc[:], func=Exp, scale=0.125)
                for half in range(2):
                    MM(
                        pv[half][:],
                        v_store[j][:, h, :],
                        pt[:, half * 512 : half * 512 + 512],
                        start=(j == 0),
                        stop=(j == 31),
                        skip_group_check=True,
                    )
            oh = opool.tile([64, TOK], bf, tag="oh", name="oh")
            o_tiles.append(oh)
            for half in range(2):
                w = wspool.tile([128, 512], f32, tag="ws", name="wst")
                nc.vector.reciprocal(out=w[64:65, :], in_=pv[half][64:65, :])
                scr = rscratch[2 * h + half : 2 * h + half + 1, :]
                dma(out=scr, in_=w[64:65, :])
                dma(out=w[0:64, :], in_=scr.partition_broadcast(64))
                nc.vector.tensor_mul(
                    out=oh[:, half * 512 : half * 512 + 512],
                    in0=pv[half][0:64, :],
                    in1=w[0:64, :],
                )

        # ---- emission ----
        proj_dout(0)
        head(0, interleave_v=True)
        head(1, interleave_v=False)
        for d in range(1, 4):
            proj_dout(d)
            head(2 * d, interleave_v=False)
            head(2 * d + 1, interleave_v=False)

        for i in range(8):
            po = projp.tile([128, 512], f32, tag="proj", name="out_ps")
            for h in range(H):
                MM(
                    po[:],
                    o_tiles[h][:, i * 128 : (i + 1) * 128],
                    wo_t[:, h, :],
                    start=(h == 0),
                    stop=False,
                    skip_group_check=True,
                )
            MM(
                po[:],
                ones1[0:1, 0:128],
                bo_t[:],
                start=False,
                stop=True,
                skip_group_check=True,
            )
            ot = ostage.tile([128, 512], f32, tag="ot", name="ot")
            nc.vector.tensor_copy(out=ot[:], in_=po[:])
            dma(out=out_p[i * 128 : (i + 1) * 128, :], in_=ot[:])

    if not nc.is_finalized():
        nc.finalize()
    return nc


def _get_program():
    global _PROGRAM
    if _PROGRAM is None:
        _PROGRAM = _build_program()
    return _PROGRAM


def _prep_inputs(q, k, v, w_q, b_q, w_k, b_k, w_v, b_v, w_o, b_o):
    bf16 = ml_dtypes.bfloat16
    q = np.asarray(q, dtype=np.float32)
    k = np.asarray(k, dtype=np.float32)
    v = np.asarray(v, dtype=np.float32)
    qT = np.ascontiguousarray(q.transpose(0, 2, 1)).astype(bf16)  # [B, D, S]
    kT = np.ascontiguousarray(k.transpose(0, 2, 1)).astype(bf16)
    vT = np.ascontiguousarray(v.transpose(0, 2, 1)).astype(bf16)
    wqT = np.ascontiguousarray(np.asarray(w_q, np.float32).T).astype(bf16)
    wkT = np.ascontiguousarray(np.asarray(w_k, np.float32).T).astype(bf16)
    wvT = np.ascontiguousarray(np.asarray(w_v, np.float32).T).astype(bf16)
    woT = np.ascontiguousarray(np.asarray(w_o, np.float32).T).astype(bf16)
    bq2 = np.asarray(b_q, np.float32).reshape(1, D).astype(bf16)
    bk2 = np.asarray(b_k, np.float32).reshape(1, D).astype(bf16)
    bv2 = np.asarray(b_v, np.float32).reshape(1, D).astype(bf16)
    bo2 = np.asarray(b_o, np.float32).reshape(1, D).astype(bf16)

    in_maps = []
    for c in range(N_CORES):
        b, s = divmod(c, 4)
        in_maps.append(
            {
                "qT": np.ascontiguousarray(qT[b][:, s * TOK : (s + 1) * TOK]),
                "kT": kT[b],
                "vT": vT[b],
                "wqT": wqT,
                "wkT": wkT,
                "wvT": wvT,
                "woT": woT,
                "bq": bq2,
                "bk": bk2,
                "bv": bv2,
                "bo": bo2,
            }
        )
    return in_maps


def run_cores(in_maps, trace=False, **kw):
    """Compile+run the SPMD program; returns BassKernelResults."""
    from concourse.bass_utils import run_bass_kernel_spmd

    nc = _get_program()
    return run_bass_kernel_spmd(nc, in_maps, list(range(N_CORES)), trace=trace, **kw)


def kernel(q, k, v, w_q, b_q, w_k, b_k, w_v, b_v, w_o, b_o):
    in_maps = _prep_inputs(q, k, v, w_q, b_q, w_k, b_k, w_v, b_v, w_o, b_o)
    res = run_cores(in_maps)
    out = np.empty((B, S, D), np.float32)
    for c in range(N_CORES):
        b, s = divmod(c, 4)
        out[b, s * TOK : (s + 1) * TOK] = res.results[c]["out"]
    return out


# revision 14
# speedup vs baseline: 1.1046x; 1.1046x over previous
"""Multi-head attention (B=2, S=4096, D=512, H=8) on 8 TRN2 NeuronCores.

Sharding: batch x sequence. Core c handles batch b=c//4, token slice
s=c%4 (1024 tokens). Each core projects the full K/V for its batch
(replicated within the 4-core batch group), projects Q for its token
slice, runs flash-style attention (scores kept transposed [tk, tq] so
no on-chip transposes are needed), and applies the output projection
for its tokens. Gather = pure concatenation, no reduction.

All matmuls in bf16 with fp32 PSUM accumulation. Softmax skips the
max-subtraction (scores ~ N(0,1); max < ~6, exp is safe in fp32) and
the denominator is obtained by augmenting V with a ones column, so
softmax costs exactly one ACT pass over the scores.
"""

import numpy as np
import ml_dtypes

B, S, D = 2, 4096, 512
H, DK = 8, 64
N_CORES = 8
TOK = B * S * DK // (N_CORES * DK)  # 1024 tokens per core
TOK = 1024

_PROGRAM = None


def _build_program():
    from contextlib import ExitStack

    import concourse.bass as bass  # noqa: F401
    import concourse.mybir as mybir
    import concourse.tile as tile
    from concourse import bacc

    bf = mybir.dt.bfloat16
    f32 = mybir.dt.float32
    Exp = mybir.ActivationFunctionType.Exp

    nc = bacc.Bacc(None)

    qT = nc.declare_dram_parameter("qT", [D, TOK], bf, isOutput=False)
    kT = nc.declare_dram_parameter("kT", [D, S], bf, isOutput=False)
    vT = nc.declare_dram_parameter("vT", [D, S], bf, isOutput=False)
    wqT = nc.declare_dram_parameter("wqT", [D, D], bf, isOutput=False)
    wkT = nc.declare_dram_parameter("wkT", [D, D], bf, isOutput=False)
    wvT = nc.declare_dram_parameter("wvT", [D, D], bf, isOutput=False)
    woT = nc.declare_dram_parameter("woT", [D, D], bf, isOutput=False)
    bq = nc.declare_dram_parameter("bq", [1, D], bf, isOutput=False)
    bk = nc.declare_dram_parameter("bk", [1, D], bf, isOutput=False)
    bv = nc.declare_dram_parameter("bv", [1, D], bf, isOutput=False)
    bo = nc.declare_dram_parameter("bo", [1, D], bf, isOutput=False)
    out_p = nc.declare_dram_parameter("out", [TOK, D], f32, isOutput=True)
    # DRAM scratch for broadcasting softmax reciprocal rows across partitions
    rscratch = nc.dram_tensor("rscratch", [H * 2, 512], f32)

    with tile.TileContext(nc) as tc, ExitStack() as ctx:
        wpool = ctx.enter_context(tc.tile_pool(name="w", bufs=1))
        kstream = ctx.enter_context(tc.tile_pool(name="kstream", bufs=2))
        qstream = ctx.enter_context(tc.tile_pool(name="qstream", bufs=2))
        ktres = ctx.enter_context(tc.tile_pool(name="ktres", bufs=4))
        qtres = ctx.enter_context(tc.tile_pool(name="qtres", bufs=4))
        vstore = ctx.enter_context(tc.tile_pool(name="vstore", bufs=32))
        ppool = ctx.enter_context(tc.tile_pool(name="p", bufs=3))
        opool = ctx.enter_context(tc.tile_pool(name="o", bufs=8))
        wspool = ctx.enter_context(tc.tile_pool(name="ws", bufs=3))
        ostage = ctx.enter_context(tc.tile_pool(name="ostage", bufs=2))
        projp = ctx.enter_context(tc.tile_pool(name="projp", bufs=2, space="PSUM"))
        scorep = ctx.enter_context(tc.tile_pool(name="scorep", bufs=2, space="PSUM"))
        pvp = ctx.enter_context(tc.tile_pool(name="pvp", bufs=2, space="PSUM"))

        dma = nc.sync.dma_start
        MM = nc.tensor.matmul

        # ---- constants: weights, biases, ones row ----
        def wtiles(param, tagp):
            t = wpool.tile([128, 4, D], bf, tag=tagp, name=tagp)
            dma(out=t[:], in_=param[:].rearrange("(c p) d -> p c d", p=128))
            return t

        wq_t = wtiles(wqT, "wq")
        wk_t = wtiles(wkT, "wk")
        wv_t = wtiles(wvT, "wv")
        wo_t = wpool.tile([DK, H, D], bf, tag="wo", name="wo_t")
        dma(out=wo_t[:], in_=woT[:].rearrange("(h p) d -> p h d", p=DK))
        bq_t = wpool.tile([1, D], bf, tag="bq", name="bq_t")
        dma(out=bq_t[:], in_=bq[:])
        bk_t = wpool.tile([1, D], bf, tag="bk", name="bk_t")
        dma(out=bk_t[:], in_=bk[:])
        bv_t = wpool.tile([1, D], bf, tag="bv", name="bv_t")
        dma(out=bv_t[:], in_=bv[:])
        bo_t = wpool.tile([1, D], bf, tag="bo", name="bo_t")
        dma(out=bo_t[:], in_=bo[:])
        ones1 = wpool.tile([1, D], bf, tag="ones", name="ones1")
        nc.vector.memset(ones1[:], 1.0)

        kt_res = []  # Q^T/K^T resident tiles, one per dout tile of 128
        qt_res = []
        v_store = []  # 32 tiles [128, H, DK+1]; last col per head = ones
        o_tiles = []  # 8 tiles [64, TOK] = normalized O_h^T

        def proj_dout(d):
            """Project Q^T and K^T for output-dim tile d (heads 2d, 2d+1)."""
            qraw = qstream.tile([128, 4, TOK], bf, tag="qraw", name="qraw")
            dma(out=qraw[:], in_=qT[:].rearrange("(c p) t -> p c t", p=128))
            qt = qtres.tile([128, TOK], bf, tag="qtres", name="qtres")
            qt_res.append(qt)
            for half in range(2):
                ps = projp.tile([128, 512], f32, tag="proj", name="proj_ps")
                for kk in range(4):
                    MM(
                        ps[:],
                        wq_t[:, kk, d * 128 : (d + 1) * 128],
                        qraw[:, kk, half * 512 : half * 512 + 512],
                        start=(kk == 0),
                        stop=False,
                        skip_group_check=True,
                    )
                MM(
                    ps[:],
                    bq_t[0:1, d * 128 : (d + 1) * 128],
                    ones1[0:1, :],
                    start=False,
                    stop=True,
                    skip_group_check=True,
                )
                nc.vector.tensor_copy(
                    out=qt[:, half * 512 : half * 512 + 512], in_=ps[:]
                )
            kt = ktres.tile([128, S], bf, tag="ktres", name="ktres")
            kt_res.append(kt)
            for tch in range(8):
                kraw = kstream.tile([128, 4, 512], bf, tag="kraw", name="kraw")
                dma(
                    out=kraw[:],
                    in_=kT[:, tch * 512 : (tch + 1) * 512].rearrange(
                        "(c p) t -> p c t", p=128
                    ),
                )
                ps = projp.tile([128, 512], f32, tag="proj", name="proj_ps")
                for kk in range(4):
                    MM(
                        ps[:],
                        wk_t[:, kk, d * 128 : (d + 1) * 128],
                        kraw[:, kk, :],
                        start=(kk == 0),
                        stop=False,
                        skip_group_check=True,
                    )
                MM(
                    ps[:],
                    bk_t[0:1, d * 128 : (d + 1) * 128],
                    ones1[0:1, :],
                    start=False,
                    stop=True,
                    skip_group_check=True,
                )
                nc.vector.tensor_copy(
                    out=kt[:, tch * 512 : (tch + 1) * 512], in_=ps[:]
                )

        vraw_cur = []

        def emit_v(j):
            """Project V for s-chunk j (tokens j*128..j*128+128)."""
            c, sub = divmod(j, 4)
            if sub == 0:
                vraw_cur.clear()
                t = kstream.tile([128, 4, 512], bf, tag="vraw", name="vraw")
                dma(
                    out=t[:],
                    in_=vT[:, c * 512 : (c + 1) * 512].rearrange(
                        "(c p) t -> p c t", p=128
                    ),
                )
                vraw_cur.append(t)
            ps = projp.tile([128, 512], f32, tag="proj", name="proj_ps")
            for kk in range(4):
                MM(
                    ps[:],
                    vraw_cur[0][:, kk, sub * 128 : (sub + 1) * 128],
                    wv_t[:, kk, :],
                    start=(kk == 0),
                    stop=False,
                    skip_group_check=True,
                )
            MM(
                ps[:],
                ones1[0:1, 0:128],
                bv_t[:],
                start=False,
                stop=True,
                skip_group_check=True,
            )
            vs = vstore.tile([128, H, DK + 1], bf, tag="vs", name="vs")
            v_store.append(vs)
            nc.vector.memset(vs[:, :, DK : DK + 1], 1.0)
            nc.vector.tensor_copy(
                out=vs[:, :, 0:DK],
                in_=ps[:].rearrange("p (h c) -> p h c", c=DK),
            )

        def head(h, interleave_v):
            kt = kt_res[h // 2]
            qt = qt_res[h // 2]
            pb = (h % 2) * 64
            pv = [pvp.tile([DK + 1, 512], f32, tag="pv", name=f"pv{_h}") for _h in range(2)]
            for j in range(32):
                if interleave_v:
                    emit_v(j)
                sc = scorep.tile([128, 1024], f32, tag="sc", name="sc")
                for half in range(2):
                    MM(
                        sc[:, half * 512 : half * 512 + 512],
                        kt[pb : pb + 64, j * 128 : (j + 1) * 128],
                        qt[pb : pb + 64, half * 512 : half * 512 + 512],
                        start=True,
                        stop=True,
                        skip_group_check=True,
                    )
                pt = ppool.tile([128, 1024], bf, tag="pt", name="pt")
                nc.scalar.activation(out=pt[:], in_=sc[:], func=Exp, scale=0.125)
                for half in range(2):
                    MM(
                        pv[half][:],
                        v_store[j][:, h, :],
                        pt[:, half * 512 : half * 512 + 512],
                        start=(j == 0),
                        stop=(j == 31),
                        skip_group_check=True,
                    )
            oh = opool.tile([64, TOK], bf, tag="oh", name="oh")
            o_tiles.append(oh)
            for half in range(2):
                # Copy PSUM accumulator out immediately so the bank frees
                # for the next head; normalize from SBUF off-critical-path.
                pvsb = wspool.tile([DK + 1, 512], f32, tag="pvsb", name="pvsb")
                nc.vector.tensor_copy(out=pvsb[:], in_=pv[half][:])
                w = wspool.tile([128, 512], f32, tag="ws", name="wst")
                nc.vector.reciprocal(out=w[64:65, :], in_=pvsb[64:65, :])
                scr = rscratch[2 * h + half : 2 * h + half + 1, :]
                dma(out=scr, in_=w[64:65, :])
                dma(out=w[0:64, :], in_=scr.partition_broadcast(64))
                nc.vector.tensor_mul(
                    out=oh[:, half * 512 : half * 512 + 512],
                    in0=pvsb[0:64, :],
                    in1=w[0:64, :],
                )

        # ---- emission ----
        proj_dout(0)
        head(0, interleave_v=True)
        head(1, interleave_v=False)
        for d in range(1, 4):
            proj_dout(d)
            head(2 * d, interleave_v=False)
            head(2 * d + 1, interleave_v=False)

        for i in range(8):
            po = projp.tile([128, 512], f32, tag="proj", name="out_ps")
            for h in range(H):
                MM(
                    po[:],
                    o_tiles[h][:, i * 128 : (i + 1) * 128],
                    wo_t[:, h, :],
                    start=(h == 0),
                    stop=False,
                    skip_group_check=True,
                )
            MM(
                po[:],
                ones1[0:1, 0:128],
                bo_t[:],
                start=False,
                stop=True,
                skip_group_check=True,
            )
            ot = ostage.tile([128, 512], f32, tag="ot", name="ot")
            nc.vector.tensor_copy(out=ot[:], in_=po[:])
            dma(out=out_p[i * 128 : (i + 1) * 128, :], in_=ot[:])

    if not nc.is_finalized():
        nc.finalize()
    return nc


def _get_program():
    global _PROGRAM
    if _PROGRAM is None:
        _PROGRAM = _build_program()
    return _PROGRAM


def _prep_inputs(q, k, v, w_q, b_q, w_k, b_k, w_v, b_v, w_o, b_o):
    bf16 = ml_dtypes.bfloat16
    q = np.asarray(q, dtype=np.float32)
    k = np.asarray(k, dtype=np.float32)
    v = np.asarray(v, dtype=np.float32)
    qT = np.ascontiguousarray(q.transpose(0, 2, 1)).astype(bf16)  # [B, D, S]
    kT = np.ascontiguousarray(k.transpose(0, 2, 1)).astype(bf16)
    vT = np.ascontiguousarray(v.transpose(0, 2, 1)).astype(bf16)
    wqT = np.ascontiguousarray(np.asarray(w_q, np.float32).T).astype(bf16)
    wkT = np.ascontiguousarray(np.asarray(w_k, np.float32).T).astype(bf16)
    wvT = np.ascontiguousarray(np.asarray(w_v, np.float32).T).astype(bf16)
    woT = np.ascontiguousarray(np.asarray(w_o, np.float32).T).astype(bf16)
    bq2 = np.asarray(b_q, np.float32).reshape(1, D).astype(bf16)
    bk2 = np.asarray(b_k, np.float32).reshape(1, D).astype(bf16)
    bv2 = np.asarray(b_v, np.float32).reshape(1, D).astype(bf16)
    bo2 = np.asarray(b_o, np.float32).reshape(1, D).astype(bf16)

    in_maps = []
    for c in range(N_CORES):
        b, s = divmod(c, 4)
        in_maps.append(
            {
                "qT": np.ascontiguousarray(qT[b][:, s * TOK : (s + 1) * TOK]),
                "kT": kT[b],
                "vT": vT[b],
                "wqT": wqT,
                "wkT": wkT,
                "wvT": wvT,
                "woT": woT,
                "bq": bq2,
                "bk": bk2,
                "bv": bv2,
                "bo": bo2,
            }
        )
    return in_maps


def run_cores(in_maps, trace=False, **kw):
    """Compile+run the SPMD program; returns BassKernelResults."""
    from concourse.bass_utils import run_bass_kernel_spmd

    nc = _get_program()
    return run_bass_kernel_spmd(nc, in_maps, list(range(N_CORES)), trace=trace, **kw)


def kernel(q, k, v, w_q, b_q, w_k, b_k, w_v, b_v, w_o, b_o):
    in_maps = _prep_inputs(q, k, v, w_q, b_q, w_k, b_k, w_v, b_v, w_o, b_o)
    res = run_cores(in_maps)
    out = np.empty((B, S, D), np.float32)
    for c in range(N_CORES):
        b, s = divmod(c, 4)
        out[b, s * TOK : (s + 1) * TOK] = res.results[c]["out"]
    return out


# revision 16
# speedup vs baseline: 1.2851x; 1.1635x over previous
"""Multi-head attention (B=2, S=4096, D=512, H=8) on 8 TRN2 NeuronCores.

Sharding: batch x sequence. Core c handles batch b=c//4, token slice
s=c%4 (1024 tokens). Each core projects the full K/V for its batch
(replicated within the 4-core batch group), projects Q for its token
slice, runs flash-style attention (scores kept transposed [tk, tq] so
no on-chip transposes are needed), and applies the output projection
for its tokens. Gather = pure concatenation, no reduction.

All matmuls in bf16 with fp32 PSUM accumulation. Softmax skips the
max-subtraction (scores ~ N(0,1); max < ~6, exp is safe in fp32) and
the denominator comes from a ones column appended to V, so softmax
costs exactly one ACT pass over the scores. The V bias is folded into
the output-projection bias (P(V+1b)/d = PV/d + b since sum(P)=d).
"""

import numpy as np
import ml_dtypes

B, S, D = 2, 4096, 512
H, DK = 8, 64
N_CORES = 8
TOK = 1024  # tokens per core

_PROGRAM = None


def _build_program():
    from contextlib import ExitStack

    import concourse.mybir as mybir
    import concourse.tile as tile
    from concourse import bacc

    bf = mybir.dt.bfloat16
    f32 = mybir.dt.float32
    Exp = mybir.ActivationFunctionType.Exp

    nc = bacc.Bacc(None)

    qT = nc.declare_dram_parameter("qT", [D, TOK], bf, isOutput=False)
    kT = nc.declare_dram_parameter("kT", [D, S], bf, isOutput=False)
    vT = nc.declare_dram_parameter("vT", [D, S], bf, isOutput=False)
    wqT = nc.declare_dram_parameter("wqT", [D, D], bf, isOutput=False)
    wkT = nc.declare_dram_parameter("wkT", [D, D], bf, isOutput=False)
    wvT = nc.declare_dram_parameter("wvT", [D, D], bf, isOutput=False)
    woT = nc.declare_dram_parameter("woT", [D, D], bf, isOutput=False)
    bq = nc.declare_dram_parameter("bq", [1, D], bf, isOutput=False)
    bk = nc.declare_dram_parameter("bk", [1, D], bf, isOutput=False)
    bvc = nc.declare_dram_parameter("bvc", [D, 1], bf, isOutput=False)
    bo = nc.declare_dram_parameter("bo", [1, D], bf, isOutput=False)
    out_p = nc.declare_dram_parameter("out", [TOK, D], f32, isOutput=True)
    # DRAM scratch rows for softmax denominator / reciprocal broadcasting
    rden = nc.dram_tensor("rden", [H * 2, 512], f32)
    rrec = nc.dram_tensor("rrec", [H * 2, 512], f32)

    with tile.TileContext(nc) as tc, ExitStack() as ctx:
        wpool = ctx.enter_context(tc.tile_pool(name="w", bufs=1))
        kstream = ctx.enter_context(tc.tile_pool(name="kstream", bufs=2))
        qstream = ctx.enter_context(tc.tile_pool(name="qstream", bufs=2))
        ktres = ctx.enter_context(tc.tile_pool(name="ktres", bufs=4))
        qtres = ctx.enter_context(tc.tile_pool(name="qtres", bufs=4))
        vstore = ctx.enter_context(tc.tile_pool(name="vstore", bufs=32))
        ppool = ctx.enter_context(tc.tile_pool(name="p", bufs=3))
        opool = ctx.enter_context(tc.tile_pool(name="o", bufs=8))
        wspool = ctx.enter_context(tc.tile_pool(name="ws", bufs=4))
        ostage = ctx.enter_context(tc.tile_pool(name="ostage", bufs=2))
        projp = ctx.enter_context(tc.tile_pool(name="projp", bufs=2, space="PSUM"))
        scorep = ctx.enter_context(tc.tile_pool(name="scorep", bufs=2, space="PSUM"))
        pvp = ctx.enter_context(tc.tile_pool(name="pvp", bufs=2, space="PSUM"))

        dma = nc.sync.dma_start
        MM = nc.tensor.matmul

        # ---- constants: weights, biases, ones row ----
        def wtiles(param, tagp):
            t = wpool.tile([128, 4, D], bf, tag=tagp, name=tagp)
            dma(out=t[:], in_=param[:].rearrange("(c p) d -> p c d", p=128))
            return t

        wq_t = wtiles(wqT, "wq")
        wk_t = wtiles(wkT, "wk")
        bq_t = wpool.tile([1, D], bf, tag="bq", name="bq_t")
        dma(out=bq_t[:], in_=bq[:])
        bk_t = wpool.tile([1, D], bf, tag="bk", name="bk_t")
        dma(out=bk_t[:], in_=bk[:])
        ones1 = wpool.tile([1, D], bf, tag="ones", name="ones1")
        nc.vector.memset(ones1[:], 1.0)
        wv_t = wtiles(wvT, "wv")
        wo_t = wpool.tile([DK, H, D], bf, tag="wo", name="wo_t")
        dma(out=wo_t[:], in_=woT[:].rearrange("(h p) d -> p h d", p=DK))
        wo2_t = wtiles(woT, "wo2")
        bvc_t = wpool.tile([128, 4, 1], bf, tag="bvc", name="bvc_t")
        dma(out=bvc_t[:], in_=bvc[:].rearrange("(c p) o -> p c o", p=128))
        bo_t = wpool.tile([1, D], bf, tag="bo", name="bo_t")
        dma(out=bo_t[:], in_=bo[:])

        # effective output bias: b_o + b_v @ w_o^T (V bias folded through)
        ps = projp.tile([128, 512], f32, tag="proj", name="boeff_ps")
        for kk in range(4):
            MM(
                ps[0:1, :],
                bvc_t[:, kk, :],
                wo2_t[:, kk, :],
                start=(kk == 0),
                stop=False,
                skip_group_check=True,
            )
        MM(
            ps[0:1, :],
            ones1[0:1, 0:1],
            bo_t[:],
            start=False,
            stop=True,
            skip_group_check=True,
        )
        boeff_t = wpool.tile([1, D], bf, tag="boeff", name="boeff_t")
        nc.vector.tensor_copy(out=boeff_t[:], in_=ps[0:1, :])

        kt_res = []  # K^T resident tiles, one per dout tile of 128
        qt_res = []  # Q^T resident tiles
        v_store = []  # 32 tiles [128, H, DK+1]; last col per head = ones
        o_tiles = []  # 8 tiles [64, TOK] = normalized O_h^T

        def proj_dout(d):
            """Project Q^T and K^T for output-dim tile d (heads 2d, 2d+1)."""
            qraw = qstream.tile([128, 4, TOK], bf, tag="qraw", name="qraw")
            dma(out=qraw[:], in_=qT[:].rearrange("(c p) t -> p c t", p=128))
            qt = qtres.tile([128, TOK], bf, tag="qtres", name="qtres")
            qt_res.append(qt)
            for half in range(2):
                ps = projp.tile([128, 512], f32, tag="proj", name="proj_ps")
                for kk in range(4):
                    MM(
                        ps[:],
                        wq_t[:, kk, d * 128 : (d + 1) * 128],
                        qraw[:, kk, half * 512 : half * 512 + 512],
                        start=(kk == 0),
                        stop=False,
                        skip_group_check=True,
                    )
                MM(
                    ps[:],
                    bq_t[0:1, d * 128 : (d + 1) * 128],
                    ones1[0:1, :],
                    start=False,
                    stop=True,
                    skip_group_check=True,
                )
                nc.vector.tensor_copy(
                    out=qt[:, half * 512 : half * 512 + 512], in_=ps[:]
                )
            kt = ktres.tile([128, S], bf, tag="ktres", name="ktres")
            kt_res.append(kt)
            for tch in range(8):
                kraw = kstream.tile([128, 4, 512], bf, tag="kraw", name="kraw")
                dma(
                    out=kraw[:],
                    in_=kT[:, tch * 512 : (tch + 1) * 512].rearrange(
                        "(c p) t -> p c t", p=128
                    ),
                )
                ps = projp.tile([128, 512], f32, tag="proj", name="proj_ps")
                for kk in range(4):
                    MM(
                        ps[:],
                        wk_t[:, kk, d * 128 : (d + 1) * 128],
                        kraw[:, kk, :],
                        start=(kk == 0),
                        stop=False,
                        skip_group_check=True,
                    )
                MM(
                    ps[:],
                    bk_t[0:1, d * 128 : (d + 1) * 128],
                    ones1[0:1, :],
                    start=False,
                    stop=True,
                    skip_group_check=True,
                )
                nc.vector.tensor_copy(
                    out=kt[:, tch * 512 : (tch + 1) * 512], in_=ps[:]
                )

        vraw_cur = []

        def emit_v(j):
            """Project V for s-chunk j (tokens j*128..j*128+128), no bias."""
            c, sub = divmod(j, 4)
            if sub == 0:
                vraw_cur.clear()
                t = kstream.tile([128, 4, 512], bf, tag="vraw", name="vraw")
                dma(
                    out=t[:],
                    in_=vT[:, c * 512 : (c + 1) * 512].rearrange(
                        "(c p) t -> p c t", p=128
                    ),
                )
                vraw_cur.append(t)
            ps = projp.tile([128, 512], f32, tag="proj", name="proj_ps")
            for kk in range(4):
                MM(
                    ps[:],
                    vraw_cur[0][:, kk, sub * 128 : (sub + 1) * 128],
                    wv_t[:, kk, :],
                    start=(kk == 0),
                    stop=(kk == 3),
                    skip_group_check=True,
                )
            vs = vstore.tile([128, H, DK + 1], bf, tag="vs", name="vs")
            v_store.append(vs)
            nc.vector.memset(vs[:, :, DK : DK + 1], 1.0)
            nc.vector.tensor_copy(
                out=vs[:, :, 0:DK],
                in_=ps[:].rearrange("p (h c) -> p h c", c=DK),
            )

        def make_norm_steps(h, pvsb, oh):
            """Closures normalizing head h's output from its SBUF copies.

            The reciprocal is computed on a [64, 8] spread of the 512
            denominators (DMA round-trip through DRAM) so it costs ~8
            DVE cycles per lane instead of 512 sequential lane-0 ones.
            """
            steps = []
            for half in range(2):
                i = 2 * h + half

                def s1(i=i, half=half, pv1=pvsb[half]):
                    dma(out=rden[i : i + 1, :], in_=pv1[64:65, :])
                    sp = wspool.tile([64, 8], f32, tag="sp", name="sp")
                    dma(out=sp[:], in_=rden[i].rearrange("(p e) -> p e", p=64))
                    sp2 = wspool.tile([64, 8], f32, tag="sp2", name="sp2")
                    nc.vector.reciprocal(out=sp2[:], in_=sp[:])
                    dma(out=rrec[i].rearrange("(p e) -> p e", p=64), in_=sp2[:])

                def s2(i=i, half=half, pv1=pvsb[half]):
                    w = wspool.tile([64, 512], f32, tag="ws", name="wst")
                    dma(out=w[:], in_=rrec[i : i + 1, :].partition_broadcast(64))
                    nc.vector.tensor_mul(
                        out=oh[:, half * 512 : half * 512 + 512],
                        in0=pv1[0:64, :],
                        in1=w[:],
                    )

                steps.append(s1)
                steps.append(s2)
            return steps

        def head(h, interleave_v, pending):
            """Attention for head h; returns normalize closures for later."""
            kt = kt_res[h // 2]
            qt = qt_res[h // 2]
            pb = (h % 2) * 64
            pv = [
                pvp.tile([DK + 1, 512], f32, tag="pv", name=f"pv{_h}")
                for _h in range(2)
            ]
            slots = {4: 0, 10: 1, 16: 2, 22: 3}
            for j in range(32):
                if interleave_v:
                    emit_v(j)
                if pending and j in slots:
                    pending[slots[j]]()
                sc = scorep.tile([128, 1024], f32, tag="sc", name="sc")
                for half in range(2):
                    MM(
                        sc[:, half * 512 : half * 512 + 512],
                        kt[pb : pb + 64, j * 128 : (j + 1) * 128],
                        qt[pb : pb + 64, half * 512 : half * 512 + 512],
                        start=True,
                        stop=True,
                        skip_group_check=True,
                    )
                pt = ppool.tile([128, 1024], bf, tag="pt", name="pt")
                nc.scalar.activation(out=pt[:], in_=sc[:], func=Exp, scale=0.125)
                for half in range(2):
                    MM(
                        pv[half][:],
                        v_store[j][:, h, :],
                        pt[:, half * 512 : half * 512 + 512],
                        start=(j == 0),
                        stop=(j == 31),
                        skip_group_check=True,
                    )
            # free the PSUM accumulators right away via SBUF copies
            pvsb = []
            for half in range(2):
                t = wspool.tile([DK + 1, 512], f32, tag="pvsb", name="pvsb")
                nc.vector.tensor_copy(out=t[:], in_=pv[half][:])
                pvsb.append(t)
            oh = opool.tile([64, TOK], bf, tag="oh", name="oh")
            o_tiles.append(oh)
            return make_norm_steps(h, pvsb, oh)

        # ---- emission ----
        proj_dout(0)
        pend = head(0, True, None)
        pend = head(1, False, pend)
        for d in range(1, 4):
            proj_dout(d)
            pend = head(2 * d, False, pend)
            pend = head(2 * d + 1, False, pend)
        for step in pend:
            step()

        for i in range(8):
            po = projp.tile([128, 512], f32, tag="proj", name="out_ps")
            for h in range(H):
                MM(
                    po[:],
                    o_tiles[h][:, i * 128 : (i + 1) * 128],
                    wo_t[:, h, :],
                    start=(h == 0),
                    stop=False,
                    skip_group_check=True,
                )
            MM(
                po[:],
                ones1[0:1, 0:128],
                boeff_t[:],
                start=False,
                stop=True,
                skip_group_check=True,
            )
            ot = ostage.tile([128, 512], f32, tag="ot", name="ot")
            nc.vector.tensor_copy(out=ot[:], in_=po[:])
            dma(out=out_p[i * 128 : (i + 1) * 128, :], in_=ot[:])

    if not nc.is_finalized():
        nc.finalize()
    return nc


def _get_program():
    global _PROGRAM
    if _PROGRAM is None:
        _PROGRAM = _build_program()
    return _PROGRAM


def _prep_inputs(q, k, v, w_q, b_q, w_k, b_k, w_v, b_v, w_o, b_o):
    bf16 = ml_dtypes.bfloat16
    q = np.asarray(q, dtype=np.float32)
    k = np.asarray(k, dtype=np.float32)
    v = np.asarray(v, dtype=np.float32)
    qT = np.ascontiguousarray(q.transpose(0, 2, 1)).astype(bf16)  # [B, D, S]
    kT = np.ascontiguousarray(k.transpose(0, 2, 1)).astype(bf16)
    vT = np.ascontiguousarray(v.transpose(0, 2, 1)).astype(bf16)
    wqT = np.ascontiguousarray(np.asarray(w_q, np.float32).T).astype(bf16)
    wkT = np.ascontiguousarray(np.asarray(w_k, np.float32).T).astype(bf16)
    wvT = np.ascontiguousarray(np.asarray(w_v, np.float32).T).astype(bf16)
    woT = np.ascontiguousarray(np.asarray(w_o, np.float32).T).astype(bf16)
    bq2 = np.asarray(b_q, np.float32).reshape(1, D).astype(bf16)
    bk2 = np.asarray(b_k, np.float32).reshape(1, D).astype(bf16)
    bv2 = np.asarray(b_v, np.float32).reshape(D, 1).astype(bf16)
    bo2 = np.asarray(b_o, np.float32).reshape(1, D).astype(bf16)

    in_maps = []
    for c in range(N_CORES):
        b, s = divmod(c, 4)
        in_maps.append(
            {
                "qT": np.ascontiguousarray(qT[b][:, s * TOK : (s + 1) * TOK]),
                "kT": kT[b],
                "vT": vT[b],
                "wqT": wqT,
                "wkT": wkT,
                "wvT": wvT,
                "woT": woT,
                "bq": bq2,
                "bk": bk2,
                "bvc": bv2,
                "bo": bo2,
            }
        )
    return in_maps


def run_cores(in_maps, trace=False, **kw):
    """Compile+run the SPMD program; returns BassKernelResults."""
    from concourse.bass_utils import run_bass_kernel_spmd

    nc = _get_program()
    return run_bass_kernel_spmd(nc, in_maps, list(range(N_CORES)), trace=trace, **kw)


def kernel(q, k, v, w_q, b_q, w_k, b_k, w_v, b_v, w_o, b_o):
    in_maps = _prep_inputs(q, k, v, w_q, b_q, w_k, b_k, w_v, b_v, w_o, b_o)
    res = run_cores(in_maps)
    out = np.empty((B, S, D), np.float32)
    for c in range(N_CORES):
        b, s = divmod(c, 4)
        out[b, s * TOK : (s + 1) * TOK] = res.results[c]["out"]
    return out


# revision 19
# speedup vs baseline: 1.3029x; 1.0138x over previous
"""Multi-head attention (B=2, S=4096, D=512, H=8) on 8 TRN2 NeuronCores.

Sharding: batch x sequence. Core c handles batch b=c//4, token slice
s=c%4 (1024 tokens). Each core projects the full K/V for its batch
(replicated within the 4-core batch group), projects Q for its token
slice, runs flash-style attention (scores kept transposed [tk, tq] so
no on-chip transposes are needed), and applies the output projection
for its tokens. Gather = pure concatenation, no reduction.

All matmuls in bf16 with fp32 PSUM accumulation. Softmax skips the
max-subtraction (scores ~ N(0,1); max < ~6, exp is safe in fp32) and
the denominator comes from a ones column appended to V, so softmax
costs exactly one ACT pass over the scores. The V bias is folded into
the output-projection bias (P(V+1b)/d = PV/d + b since sum(P)=d).

Pipelining: scores for chunk u+1 are emitted before PV of chunk u so
the tensor engine computes them while ACT runs exp(u); head h's
normalization (reciprocal via a [64,8] DMA-spread) is emitted inside
head h+1's loop; the output projection for heads 0-3 is accumulated
into SBUF during heads 6-7, leaving only heads 4-7 for the tail.
"""

import numpy as np
import ml_dtypes

B, S, D = 2, 4096, 512
H, DK = 8, 64
N_CORES = 8
TOK = 1024  # tokens per core

_PROGRAM = None


def _build_program():
    from contextlib import ExitStack

    import concourse.mybir as mybir
    import concourse.tile as tile
    from concourse import bacc

    bf = mybir.dt.bfloat16
    f32 = mybir.dt.float32
    Exp = mybir.ActivationFunctionType.Exp

    nc = bacc.Bacc(None)

    qT = nc.declare_dram_parameter("qT", [D, TOK], bf, isOutput=False)
    kT = nc.declare_dram_parameter("kT", [D, S], bf, isOutput=False)
    vT = nc.declare_dram_parameter("vT", [D, S], bf, isOutput=False)
    wqT = nc.declare_dram_parameter("wqT", [D, D], bf, isOutput=False)
    wkT = nc.declare_dram_parameter("wkT", [D, D], bf, isOutput=False)
    wvT = nc.declare_dram_parameter("wvT", [D, D], bf, isOutput=False)
    woT = nc.declare_dram_parameter("woT", [D, D], bf, isOutput=False)
    bq = nc.declare_dram_parameter("bq", [1, D], bf, isOutput=False)
    bk = nc.declare_dram_parameter("bk", [1, D], bf, isOutput=False)
    bvc = nc.declare_dram_parameter("bvc", [D, 1], bf, isOutput=False)
    bo = nc.declare_dram_parameter("bo", [1, D], bf, isOutput=False)
    out_p = nc.declare_dram_parameter("out", [TOK, D], f32, isOutput=True)
    # DRAM scratch rows for softmax denominator / reciprocal broadcasting
    rden = nc.dram_tensor("rden", [H * 2, 512], f32)
    rrec = nc.dram_tensor("rrec", [H * 2, 512], f32)

    with tile.TileContext(nc) as tc, ExitStack() as ctx:
        wpool = ctx.enter_context(tc.tile_pool(name="w", bufs=1))
        kstream = ctx.enter_context(tc.tile_pool(name="kstream", bufs=2))
        vstream = ctx.enter_context(tc.tile_pool(name="vstream", bufs=3))
        qstream = ctx.enter_context(tc.tile_pool(name="qstream", bufs=2))
        ktres = ctx.enter_context(tc.tile_pool(name="ktres", bufs=4))
        qtres = ctx.enter_context(tc.tile_pool(name="qtres", bufs=4))
        vstore = ctx.enter_context(tc.tile_pool(name="vstore", bufs=32))
        ppool = ctx.enter_context(tc.tile_pool(name="p", bufs=3))
        opool = ctx.enter_context(tc.tile_pool(name="o", bufs=8))
        oaccp = ctx.enter_context(tc.tile_pool(name="oacc", bufs=8))
        wspool = ctx.enter_context(tc.tile_pool(name="ws", bufs=4))
        ostage = ctx.enter_context(tc.tile_pool(name="ostage", bufs=2))
        projp = ctx.enter_context(tc.tile_pool(name="projp", bufs=2, space="PSUM"))
        scorep = ctx.enter_context(tc.tile_pool(name="scorep", bufs=2, space="PSUM"))
        pvp = ctx.enter_context(tc.tile_pool(name="pvp", bufs=2, space="PSUM"))

        dma = nc.sync.dma_start
        MM = nc.tensor.matmul

        # ---- essential constants first (everything else is deferred) ----
        def wtiles(param, tagp):
            t = wpool.tile([128, 4, D], bf, tag=tagp, name=tagp)
            dma(out=t[:], in_=param[:].rearrange("(c p) d -> p c d", p=128))
            return t

        wq_t = wtiles(wqT, "wq")
        wk_t = wtiles(wkT, "wk")
        bq_t = wpool.tile([1, D], bf, tag="bq", name="bq_t")
        dma(out=bq_t[:], in_=bq[:])
        bk_t = wpool.tile([1, D], bf, tag="bk", name="bk_t")
        dma(out=bk_t[:], in_=bk[:])
        ones1 = wpool.tile([1, D], bf, tag="ones", name="ones1")
        nc.vector.memset(ones1[:], 1.0)

        kt_res = []  # K^T resident tiles, one per dout tile of 128
        qt_res = []  # Q^T resident tiles
        v_store = []  # 32 tiles [128, H, DK+1]; last col per head = ones
        o_tiles = []  # 8 tiles [64, TOK] = normalized O_h^T
        oacc_tiles = []  # 8 tiles [128, 512] f32: out-proj partials (heads 0-3)

        def proj_q_half(d, half, qraw, qt):
            ps = projp.tile([128, 512], f32, tag="proj", name="proj_ps")
            for kk in range(4):
                MM(
                    ps[:],
                    wq_t[:, kk, d * 128 : (d + 1) * 128],
                    qraw[:, kk, half * 512 : half * 512 + 512],
                    start=(kk == 0),
                    stop=False,
                    skip_group_check=True,
                )
            MM(
                ps[:],
                bq_t[0:1, d * 128 : (d + 1) * 128],
                ones1[0:1, :],
                start=False,
                stop=True,
                skip_group_check=True,
            )
            nc.vector.tensor_copy(
                out=qt[:, half * 512 : half * 512 + 512], in_=ps[:]
            )

        def proj_k_chunk(d, tch, kt):
            kraw = kstream.tile([128, 4, 512], bf, tag="kraw", name="kraw")
            dma(
                out=kraw[:],
                in_=kT[:, tch * 512 : (tch + 1) * 512].rearrange(
                    "(c p) t -> p c t", p=128
                ),
            )
            ps = projp.tile([128, 512], f32, tag="proj", name="proj_ps")
            for kk in range(4):
                MM(
                    ps[:],
                    wk_t[:, kk, d * 128 : (d + 1) * 128],
                    kraw[:, kk, :],
                    start=(kk == 0),
                    stop=False,
                    skip_group_check=True,
                )
            MM(
                ps[:],
                bk_t[0:1, d * 128 : (d + 1) * 128],
                ones1[0:1, :],
                start=False,
                stop=True,
                skip_group_check=True,
            )
            nc.vector.tensor_copy(
                out=kt[:, tch * 512 : (tch + 1) * 512], in_=ps[:]
            )

        def make_proj_closures(d):
            """10 closures projecting Q^T/K^T for dout tile d. The qraw
            load is deferred into the first closure so its DMA is not
            queued (and WAR-blocked) long before it can run."""
            qt = qtres.tile([128, TOK], bf, tag="qtres", name="qtres")
            qt_res.append(qt)
            kt = ktres.tile([128, S], bf, tag="ktres", name="ktres")
            kt_res.append(kt)
            box = {}

            def q_first():
                qraw = qstream.tile([128, 4, TOK], bf, tag="qraw", name="qraw")
                dma(out=qraw[:], in_=qT[:].rearrange("(c p) t -> p c t", p=128))
                box["qraw"] = qraw
                proj_q_half(d, 0, qraw, qt)

            cl = [q_first, lambda: proj_q_half(d, 1, box["qraw"], qt)]
            cl += [lambda tch=tch: proj_k_chunk(d, tch, kt) for tch in range(8)]
            return cl

        def proj_dout(d):
            for fn in make_proj_closures(d):
                fn()

        # ---- deferred constant loads / computations (closures) ----
        wv_t = None
        vraw_tiles = {}

        def load_wv():
            nonlocal wv_t
            wv_t = wtiles(wvT, "wv")

        def load_vraw(c):
            t = vstream.tile([128, 4, 512], bf, tag="vraw", name="vraw")
            dma(
                out=t[:],
                in_=vT[:, c * 512 : (c + 1) * 512].rearrange(
                    "(c p) t -> p c t", p=128
                ),
            )
            vraw_tiles[c] = t

        wo_t = wo2_t = bvc_t = bo_t = boeff_t = None

        def load_wo():
            nonlocal wo_t, wo2_t, bvc_t, bo_t
            wo_t = wpool.tile([DK, H, D], bf, tag="wo", name="wo_t")
            dma(out=wo_t[:], in_=woT[:].rearrange("(h p) d -> p h d", p=DK))
            wo2_t = wtiles(woT, "wo2")
            bvc_t = wpool.tile([128, 4, 1], bf, tag="bvc", name="bvc_t")
            dma(out=bvc_t[:], in_=bvc[:].rearrange("(c p) o -> p c o", p=128))
            bo_t = wpool.tile([1, D], bf, tag="bo", name="bo_t")
            dma(out=bo_t[:], in_=bo[:])

        def emit_boeff():
            """b_o + b_v @ w_o^T (V bias folded through the out projection)."""
            nonlocal boeff_t
            ps = projp.tile([128, 512], f32, tag="proj", name="boeff_ps")
            for kk in range(4):
                MM(
                    ps[0:1, :],
                    bvc_t[:, kk, :],
                    wo2_t[:, kk, :],
                    start=(kk == 0),
                    stop=False,
                    skip_group_check=True,
                )
            MM(
                ps[0:1, :],
                ones1[0:1, 0:1],
                bo_t[:],
                start=False,
                stop=True,
                skip_group_check=True,
            )
            boeff_t = wpool.tile([1, D], bf, tag="boeff", name="boeff_t")
            nc.vector.tensor_copy(out=boeff_t[:], in_=ps[0:1, :])

        def emit_v(j):
            """Project V for s-chunk j (tokens j*128..j*128+128), no bias."""
            c, sub = divmod(j, 4)
            if sub == 0 and c + 1 < 8:
                load_vraw(c + 1)
            ps = projp.tile([128, 512], f32, tag="proj", name="proj_ps")
            for kk in range(4):
                MM(
                    ps[:],
                    vraw_tiles[c][:, kk, sub * 128 : (sub + 1) * 128],
                    wv_t[:, kk, :],
                    start=(kk == 0),
                    stop=(kk == 3),
                    skip_group_check=True,
                )
            vs = vstore.tile([128, H, DK + 1], bf, tag="vs", name="vs")
            v_store.append(vs)
            nc.vector.memset(vs[:, :, DK : DK + 1], 1.0)
            nc.vector.tensor_copy(
                out=vs[:, :, 0:DK],
                in_=ps[:].rearrange("p (h c) -> p h c", c=DK),
            )

        def emit_oacc(i):
            """Out-projection partial for tq-tile i over heads 0-3 -> SBUF."""
            ps = projp.tile([128, 512], f32, tag="proj", name="oacc_ps")
            for h in range(4):
                MM(
                    ps[:],
                    o_tiles[h][:, i * 128 : (i + 1) * 128],
                    wo_t[:, h, :],
                    start=(h == 0),
                    stop=(h == 3),
                    skip_group_check=True,
                )
            t = oaccp.tile([128, 512], f32, tag="oacc", name="oacc")
            oacc_tiles.append(t)
            nc.vector.tensor_copy(out=t[:], in_=ps[:])

        def make_norm_steps(h, pvsb, oh):
            """Closures normalizing head h's output from its SBUF copies."""
            steps = []
            for half in range(2):
                i = 2 * h + half

                def s1(i=i, pv1=pvsb[half]):
                    dma(out=rden[i : i + 1, :], in_=pv1[64:65, :])
                    sp = wspool.tile([64, 8], f32, tag="sp", name="sp")
                    dma(out=sp[:], in_=rden[i].rearrange("(p e) -> p e", p=64))
                    sp2 = wspool.tile([64, 8], f32, tag="sp2", name="sp2")
                    nc.vector.reciprocal(out=sp2[:], in_=sp[:])
                    dma(out=rrec[i].rearrange("(p e) -> p e", p=64), in_=sp2[:])

                def s2(i=i, half=half, pv1=pvsb[half]):
                    w = wspool.tile([64, 512], f32, tag="ws", name="wst")
                    dma(out=w[:], in_=rrec[i : i + 1, :].partition_broadcast(64))
                    nc.vector.tensor_mul(
                        out=oh[:, half * 512 : half * 512 + 512],
                        in0=pv1[0:64, :],
                        in1=w[:],
                    )

                steps.append(s1)
                steps.append(s2)
            return steps

        # ---- attention: flat unit stream, scores one chunk ahead ----
        def emit_scores(h, j):
            kt = kt_res[h // 2]
            qt = qt_res[h // 2]
            pb = (h % 2) * 64
            sc = scorep.tile([128, 1024], f32, tag="sc", name="sc")
            for half in range(2):
                MM(
                    sc[:, half * 512 : half * 512 + 512],
                    kt[pb : pb + 64, j * 128 : (j + 1) * 128],
                    qt[pb : pb + 64, half * 512 : half * 512 + 512],
                    start=True,
                    stop=True,
                    skip_group_check=True,
                )
            return sc

        load_wv()
        load_vraw(0)
        proj_dout(0)
        emit_v(0)

        # per-unit extra-work schedule: (head, chunk) -> list of closures
        extra = {}
        extra[(1, 2)] = [load_wo]
        extra[(1, 30)] = [emit_boeff]
        # phase-d Q/K projections spread over the two preceding heads
        d1 = make_proj_closures(1)
        for n, jj in enumerate((1, 5, 8, 11, 14, 17, 20, 23, 26, 29)):
            extra.setdefault((1, jj), []).append(d1[n])
        for d in (2, 3):
            cls = make_proj_closures(d)
            for n, jj in enumerate((1, 7, 13, 19, 25)):
                extra.setdefault((2 * d - 2, jj), []).append(cls[n])
                extra.setdefault((2 * d - 1, jj), []).append(cls[n + 5])
        pend_norm = {}  # filled as heads complete

        units = [(h, j) for h in range(H) for j in range(32)]
        pv_tiles = {}
        pvsb_tiles = {}
        norm_slots = {4: 0, 10: 1, 16: 2, 22: 3}
        oacc_slots = {}
        for i in range(8):
            hh = 6 + i // 4
            jj = 2 + 6 * (i % 4)
            oacc_slots.setdefault((hh, jj), []).append(i)

        sc_next = emit_scores(0, 0)
        for idx, (h, j) in enumerate(units):
            if j == 0:
                pv_tiles[h] = [
                    pvp.tile([DK + 1, 512], f32, tag="pv", name=f"pv{_h}")
                    for _h in range(2)
                ]
            sc = sc_next
            pt = ppool.tile([128, 1024], bf, tag="pt", name="pt")
            nc.scalar.activation(out=pt[:], in_=sc[:], func=Exp, scale=0.125)
            # interleaved extra work (runs on PE/DVE/DMA while ACT is busy)
            if h == 0 and j + 1 < 32:
                emit_v(j + 1)
            for fn in extra.get((h, j), ()):
                fn()
            if h >= 1 and j in norm_slots and (h - 1) in pend_norm:
                pend_norm[h - 1][norm_slots[j]]()
            for i in oacc_slots.get((h, j), ()):
                emit_oacc(i)
            # next unit's scores go to PE before this unit's PV
            if idx + 1 < len(units):
                nh, nj = units[idx + 1]
                sc_next = emit_scores(nh, nj)
            pv = pv_tiles[h]
            for half in range(2):
                MM(
                    pv[half][:],
                    v_store[j][:, h, :],
                    pt[:, half * 512 : half * 512 + 512],
                    start=(j == 0),
                    stop=(j == 31),
                    skip_group_check=True,
                )
            if j == 31:
                pvsb = []
                for half in range(2):
                    t = wspool.tile([DK + 1, 512], f32, tag="pvsb", name="pvsb")
                    nc.vector.tensor_copy(out=t[:], in_=pv[half][:])
                    pvsb.append(t)
                pvsb_tiles[h] = pvsb
                oh = opool.tile([64, TOK], bf, tag="oh", name="oh")
                o_tiles.append(oh)
                pend_norm[h] = make_norm_steps(h, pvsb, oh)

        # head 7's normalization (nothing left to interleave with)
        for step in pend_norm[7]:
            step()

        # ---- output projection tail: heads 4-7 + folded bias + partials ----
        for i in range(8):
            po = projp.tile([128, 512], f32, tag="proj", name="out_ps")
            for h in range(4, 8):
                MM(
                    po[:],
                    o_tiles[h][:, i * 128 : (i + 1) * 128],
                    wo_t[:, h, :],
                    start=(h == 4),
                    stop=False,
                    skip_group_check=True,
                )
            MM(
                po[:],
                ones1[0:1, 0:128],
                boeff_t[:],
                start=False,
                stop=True,
                skip_group_check=True,
            )
            ot = ostage.tile([128, 512], f32, tag="ot", name="ot")
            nc.vector.tensor_add(out=ot[:], in0=po[:], in1=oacc_tiles[i][:])
            dma(out=out_p[i * 128 : (i + 1) * 128, :], in_=ot[:])

    if not nc.is_finalized():
        nc.finalize()
    return nc


def _get_program():
    global _PROGRAM
    if _PROGRAM is None:
        _PROGRAM = _build_program()
    return _PROGRAM


def _prep_inputs(q, k, v, w_q, b_q, w_k, b_k, w_v, b_v, w_o, b_o):
    bf16 = ml_dtypes.bfloat16
    q = np.asarray(q, dtype=np.float32)
    k = np.asarray(k, dtype=np.float32)
    v = np.asarray(v, dtype=np.float32)
    qT = np.ascontiguousarray(q.transpose(0, 2, 1)).astype(bf16)  # [B, D, S]
    kT = np.ascontiguousarray(k.transpose(0, 2, 1)).astype(bf16)
    vT = np.ascontiguousarray(v.transpose(0, 2, 1)).astype(bf16)
    wqT = np.ascontiguousarray(np.asarray(w_q, np.float32).T).astype(bf16)
    wkT = np.ascontiguousarray(np.asarray(w_k, np.float32).T).astype(bf16)
    wvT = np.ascontiguousarray(np.asarray(w_v, np.float32).T).astype(bf16)
    woT = np.ascontiguousarray(np.asarray(w_o, np.float32).T).astype(bf16)
    bq2 = np.asarray(b_q, np.float32).reshape(1, D).astype(bf16)
    bk2 = np.asarray(b_k, np.float32).reshape(1, D).astype(bf16)
    bv2 = np.asarray(b_v, np.float32).reshape(D, 1).astype(bf16)
    bo2 = np.asarray(b_o, np.float32).reshape(1, D).astype(bf16)

    in_maps = []
    for c in range(N_CORES):
        b, s = divmod(c, 4)
        in_maps.append(
            {
                "qT": np.ascontiguousarray(qT[b][:, s * TOK : (s + 1) * TOK]),
                "kT": kT[b],
                "vT": vT[b],
                "wqT": wqT,
                "wkT": wkT,
                "wvT": wvT,
                "woT": woT,
                "bq": bq2,
                "bk": bk2,
                "bvc": bv2,
                "bo": bo2,
            }
        )
    return in_maps


def run_cores(in_maps, trace=False, **kw):
    """Compile+run the SPMD program; returns BassKernelResults."""
    from concourse.bass_utils import run_bass_kernel_spmd

    nc = _get_program()
    return run_bass_kernel_spmd(nc, in_maps, list(range(N_CORES)), trace=trace, **kw)


def kernel(q, k, v, w_q, b_q, w_k, b_k, w_v, b_v, w_o, b_o):
    in_maps = _prep_inputs(q, k, v, w_q, b_q, w_k, b_k, w_v, b_v, w_o, b_o)
    res = run_cores(in_maps)
    out = np.empty((B, S, D), np.float32)
    for c in range(N_CORES):
        b, s = divmod(c, 4)
        out[b, s * TOK : (s + 1) * TOK] = res.results[c]["out"]
    return out
